# revision 1
# baseline (speedup 1.0000x reference)
"""Conv2dfft forward on 8 TRN2 NeuronCores.

The reference computes cross-correlation via rfft2/irfft2 on a 65x65 grid.
Because the FFT grid (65) >= padded_H + KH - 1 (34 + 3 - 1 = 36 would wrap,
but the output is cropped to out_H=32 and max input row touched is 33 < 65),
no circular wraparound reaches the cropped output: the result is EXACTLY a
3x3 same-padding cross-correlation (DL conv, padding=1) plus bias.

So: direct conv as 9 PSUM-accumulated matmuls per output tile.
  out[f, (y,x)] = sum_t sum_c wT[c, t, f] * xpad[c, y+dy_t, x+dx_t]
with contraction over C=128 on the partition dim.

Sharding: data-parallel over batch N=32 -> 4 images per core.

Schedule (HW-tuned): one whole-image input DMA per image on the SP ring
(V2=0) — finer row-piece DMAs serialize ~1us/op on the HWDGE ring and
gate the PE; weights+bias+outputs ride the ACT ring. Per-core PE floor
is 72-90 matmuls x 512 moving cols at 2.4GHz ~= 15.4us; measured loop
iteration ~19.9us (ramp + drain + loop barrier account for the rest).
"""

import os
from contextlib import ExitStack

import numpy as np

import concourse.bacc as bacc
import concourse.mybir as mybir
import concourse.tile as tile
from concourse import bass_utils

N_CORES = 8
N, C, H, W = 32, 128, 32, 32
F = 128
KH = KW = 3
PH, PW = H + 2, W + 2          # padded input 34x34
NLOC = N // N_CORES            # images per core
# output rows per chunk; 16 -> CH*W = 512 = one full PSUM bank
CH = int(os.environ.get("CONV_CH", "16"))
NCHUNK = H // CH

# matmul input dtype for x and w. fp16 runs the PE at full rate (1 cycle/row,
# like bf16) with a 10-bit mantissa: HW-measured rel err 3.2e-4 vs the fp32
# reference. ("f32r" = 1.4e-4 but ~6% slower; "f32" exact but 4x slower.)
MM_DT = os.environ.get("CONV_MM_DT", "fp16")

# Number of times the compute body is emitted in the NEFF (timing tool:
# per-iteration HW time = (t_K - t_1) / (K - 1), host-RPC overhead cancels).
N_ITERS = int(os.environ.get("CONV_ITERS", "1"))

# v2 pipelining: split input DMAs into row-pieces + PE warmup matmuls.
# Default OFF: HWDGE processes DMA ops on a ring FIFO with ~1us of
# per-op completion serialization, so 9 small pieces gate the PE ~5us
# worse than 4 whole-image DMAs (HW-measured: 25.0us -> 19.9us).
V2 = bool(int(os.environ.get("CONV_V2", "0")))
N_WARMUP_MM = int(os.environ.get("CONV_WARMUP", "4"))

# diagnostics: selectively disable pipeline stages (timing only — results wrong)
SKIP_IN = bool(int(os.environ.get("CONV_SKIP_IN", "0")))
SKIP_OUT = bool(int(os.environ.get("CONV_SKIP_OUT", "0")))
SKIP_MM = bool(int(os.environ.get("CONV_SKIP_MM", "0")))
# split the weight DMA into one piece per tap so the first matmul only waits
# for tap 0 (32KB) instead of the whole 295KB weight block
WSPLIT = bool(int(os.environ.get("CONV_WSPLIT", "0")))
# alternate output DMAs between the ACT and SP HWDGE rings (SP is idle once
# the input stream finishes, so this halves the per-ring output load)
ORING = bool(int(os.environ.get("CONV_ORING", "0")))
# contiguous-rhs schedule: keep x in SBUF as a flat [C, PH*PW] tile and
# stream whole padded rows (rows*34 cols incl. 2 junk cols/row) per matmul.
# Every tap is then a pure start-offset into one flat contiguous AP (no
# nested row-strided access pattern on the PE moving operand); DVE copies
# out only the 32 valid columns per row.
CONTIG = bool(int(os.environ.get("CONV_CONTIG", "0")))

_DT_MAP = {
    "f32r": mybir.dt.float32r,
    "f32": mybir.dt.float32,
    "bf16": mybir.dt.bfloat16,
    "fp16": mybir.dt.float16,
}

_cached_nc = None
LAST_RESULT = None


def _build():
    dt_mm = _DT_MAP[MM_DT]
    nc = bacc.Bacc(
        "TRN2",
        target_bir_lowering=False,
        debug=False,
        num_devices=N_CORES,
    )

    xp = nc.dram_tensor("xp", [NLOC, C, PH, PW], dt_mm, kind="ExternalInput")
    wt = nc.dram_tensor("wt", [C, KH * KW * F], dt_mm, kind="ExternalInput")
    bb = nc.dram_tensor("bb", [F, 1], mybir.dt.float32, kind="ExternalInput")
    out = nc.dram_tensor("out", [NLOC, F, H, W], mybir.dt.float32, kind="ExternalOutput")

    xp_ap = xp.ap()
    out_ap = out.ap()

    if bool(int(os.environ.get("CONV_NULL", "0"))):
        # dispatch-floor measurement: one tiny DMA through SBUF
        with ExitStack() as ctx:
            tc = ctx.enter_context(tile.TileContext(nc))
            np_pool = ctx.enter_context(tc.tile_pool(name="np", bufs=1))
            t_sb = np_pool.tile([F, 1], mybir.dt.float32)
            nc.sync.dma_start(t_sb[:], bb.ap())
            nc.sync.dma_start(out_ap[0, :, 0:1, 0:1], t_sb[:])
        nc.compile()
        return nc

    with ExitStack() as ctx:
        tc = ctx.enter_context(tile.TileContext(nc))
        const_pool = ctx.enter_context(tc.tile_pool(name="const", bufs=1))
        x_pool = ctx.enter_context(
            tc.tile_pool(name="xs", bufs=int(os.environ.get("CONV_XBUFS", "4")))
        )
        _loopwarm = int(os.environ.get("CONV_LOOPWARM", "0"))
        n_ps_bufs = 7 if ((V2 and N_WARMUP_MM > 0) or _loopwarm > 0) else 8
        ps_pool = ctx.enter_context(tc.tile_pool(name="ps", bufs=n_ps_bufs, space="PSUM"))
        o_pool = ctx.enter_context(
            tc.tile_pool(name="os", bufs=int(os.environ.get("CONV_OBUFS", "5")))
        )

        # weights + bias arrive via the ACT HWDGE ring so they don't delay
        # the image stream on the SP ring
        wt_sb = const_pool.tile([C, KH * KW * F], dt_mm)
        if WSPLIT:
            for t in range(KH * KW):
                nc.scalar.dma_start(
                    wt_sb[:, t * F : (t + 1) * F], wt.ap()[:, t * F : (t + 1) * F]
                )
        else:
            nc.scalar.dma_start(wt_sb[:], wt.ap())
        b_sb = const_pool.tile([F, 1], mybir.dt.float32)
        nc.scalar.dma_start(b_sb[:], bb.ap())

        LOOPWARM = _loopwarm
        wps_pool = warm_sb = warm_out = None
        if (V2 and N_WARMUP_MM > 0) or LOOPWARM > 0:
            # HAM warmup: keep PE busy while the first input DMAs land.
            wps_pool = ctx.enter_context(
                tc.tile_pool(name="wps", bufs=1, space="PSUM")
            )
            warm_sb = const_pool.tile([C, F], mybir.dt.float32)
            nc.vector.memset(warm_sb[:], 0.0)
            warm_out = const_pool.tile([F, 1], mybir.dt.float32)
        if V2 and N_WARMUP_MM > 0:
            wps = wps_pool.tile([F, F], mybir.dt.float32)
            for j in range(N_WARMUP_MM):
                nc.tensor.matmul(
                    wps[:],
                    warm_sb[:],
                    warm_sb[:],
                    start=(j == 0),
                    stop=(j == N_WARMUP_MM - 1),
                )
            nc.vector.tensor_copy(warm_out[:], wps[:, 0:1])

        TAPER = bool(int(os.environ.get("CONV_TAPER", "1")))

        XCONST = bool(int(os.environ.get("CONV_XCONST", "0")))
        IRING = int(os.environ.get("CONV_IRING", "1"))
        RAMPSPLIT = bool(int(os.environ.get("CONV_RAMPSPLIT", "0")))

        x_const = None
        if SKIP_IN or XCONST:
            # PE-isolation probe: all chunks read one memset tile; no input
            # DMAs inside the timed loop
            x_const = const_pool.tile([C, PH, PW], dt_mm)
            nc.vector.memset(x_const[:], 0.0)

        def image_plan(n):
            # (out-row chunks, input DMA row-splits). Tapered: the very first
            # chunk is 8 rows (compute starts after a third of image 0) and
            # the very last is 8 rows (shorter serial drain tail).
            if CH != 16:
                # generic chunking probe: vary matmul granularity while
                # keeping the input DMA split fixed at 18+16 rows
                chunks = [(i * CH, CH) for i in range(H // CH)]
                return chunks, [(0, 18), (18, PH)]
            if TAPER and n == 0:
                return [(0, 8), (8, 8), (16, 16)], [(0, 10), (10, 18), (18, PH)]
            if TAPER and n == NLOC - 1:
                return [(0, 16), (16, 8), (24, 8)], [(0, CH + 2), (CH + 2, PH)]
            return [(0, CH), (CH, CH)], [(0, CH + 2), (CH + 2, PH)]

        INTERLEAVE = bool(int(os.environ.get("CONV_INTERLEAVE", "0")))
        chunk_counter = [0]

        def emit_chunk(x_sb, n, y0, rows):
            if SKIP_MM:
                return
            ci = chunk_counter[0]
            chunk_counter[0] += 1
            ps = ps_pool.tile([F, rows, W], mybir.dt.float32, tag="ps")
            t = 0
            for dy in range(KH):
                for dx in range(KW):
                    rhs = x_sb[:, y0 + dy : y0 + dy + rows, dx : dx + W]
                    nc.tensor.matmul(
                        ps[:],
                        wt_sb[:, t * F : (t + 1) * F],
                        rhs,
                        start=(t == 0),
                        stop=(t == KH * KW - 1),
                    )
                    t += 1
            o_sb = o_pool.tile([F, rows, W], mybir.dt.float32, tag="os")
            nc.vector.tensor_scalar_add(o_sb[:], ps[:], b_sb[:])
            if not SKIP_OUT:
                eng = nc.sync if (ORING and ci % 2 == 1) else nc.scalar
                eng.dma_start(out_ap[n, :, y0 : y0 + rows, :], o_sb[:])

        def body():
            if LOOPWARM > 0:
                # per-iteration PE filler: covers the image-0 DMA ramp so the
                # PE has no idle window at the loop back-edge
                wps = wps_pool.tile([F, F], mybir.dt.float32)
                for j in range(LOOPWARM):
                    nc.tensor.matmul(
                        wps[:],
                        warm_sb[:],
                        warm_sb[:],
                        start=(j == 0),
                        stop=(j == LOOPWARM - 1),
                    )
                nc.vector.tensor_copy(warm_out[:], wps[:, 0:1])
            if not (V2 and INTERLEAVE):
                for n in range(NLOC):
                    chunks_n, dma_splits = image_plan(n)
                    if SKIP_IN:
                        for y0, rows in chunks_n:
                            emit_chunk(x_const, n, y0, rows)
                        continue
                    x_sb = x_pool.tile([C, PH, PW], dt_mm, tag="xs")
                    if V2:
                        for pi, (r0, r1) in enumerate(dma_splits):
                            eng = nc.scalar if (IRING == 2 and pi % 2) else nc.sync
                            eng.dma_start(
                                x_sb[:, r0:r1, :], xp_ap[n, :, r0:r1, :]
                            )
                    else:
                        eng = nc.scalar if (IRING == 2 and n % 2) else nc.sync
                        if RAMPSPLIT and n == 0:
                            # split image 0 so the PE can start ~0.8us earlier
                            eng.dma_start(
                                x_sb[:, 0:18, :], xp_ap[n, :, 0:18, :]
                            )
                            eng.dma_start(
                                x_sb[:, 18:PH, :], xp_ap[n, :, 18:PH, :]
                            )
                        else:
                            eng.dma_start(x_sb[:], xp_ap[n, :, :, :])
                    for y0, rows in chunks_n:
                        emit_chunk(x_const if XCONST else x_sb, n, y0, rows)
                return

            # interleaved schedule: issue DMA pieces in the order chunks need
            # them and interleave chunk compute across images, so the PE works
            # on image n+1's first chunk while image n's tail DMA lands.
            x_tiles = [
                x_pool.tile([C, PH, PW], dt_mm, tag="xs", name=f"xt{i}")
                for i in range(NLOC)
            ]
            plans = [image_plan(n) for n in range(NLOC)]
            # (image, piece) DMA order / (image, chunk) compute order for
            # NLOC=4 with the tapered plans (3,2,2,3 chunks; 3,2,2,2 pieces)
            dma_order = [
                (0, 0), (0, 1), (1, 0), (0, 2), (1, 1),
                (2, 0), (2, 1), (3, 0), (3, 1),
            ]
            chunk_order = [
                (0, 0), (0, 1), (1, 0), (0, 2), (1, 1),
                (2, 0), (2, 1), (3, 0), (3, 1), (3, 2),
            ]
            emitted = set()
            ci = 0
            for di, (n, p) in enumerate(dma_order):
                r0, r1 = plans[n][1][p]
                nc.sync.dma_start(
                    x_tiles[n][:, r0:r1, :], xp_ap[n, :, r0:r1, :]
                )
                emitted.add((n, p))
                # emit every chunk whose input pieces have all been issued
                while ci < len(chunk_order):
                    cn, cc = chunk_order[ci]
                    y0, rows = plans[cn][0][cc]
                    need = [
                        pi
                        for pi, (pr0, pr1) in enumerate(plans[cn][1])
                        if pr0 < y0 + rows + 2 and pr1 > y0
                    ]
                    if not all((cn, pi) in emitted for pi in need):
                        break
                    emit_chunk(x_tiles[cn], cn, y0, rows)
                    ci += 1
            while ci < len(chunk_order):
                cn, cc = chunk_order[ci]
                y0, rows = plans[cn][0][cc]
                emit_chunk(x_tiles[cn], cn, y0, rows)
                ci += 1

        unroll = int(os.environ.get("CONV_UNROLL", "1"))
        if N_ITERS == 1:
            for _ in range(unroll):
                body()
        else:
            # timing mode: run the body N_ITERS*UNROLL times on-device
            stag = bool(int(os.environ.get("CONV_STAGGER", "0")))
            with tc.For_i(0, N_ITERS, 1, staggered_reset=stag):
                for _ in range(unroll):
                    body()

    nc.compile()
    return nc


def _build_raw():
    """Raw-Bacc variant: hand-rolled semaphores, no Tile framework.

    Exactly 8 output chunks per core -> each chunk owns one PSUM bank, all
    SBUF statically allocated (no reuse, no WAR hazards). Avoids the Tile
    kernel-tail drain + all-engine barrier.
    """
    dt_mm = _DT_MAP[MM_DT]
    nc = bacc.Bacc(
        "TRN2",
        target_bir_lowering=False,
        debug=False,
        num_devices=N_CORES,
    )

    xp = nc.dram_tensor("xp", [NLOC, C, PH, PW], dt_mm, kind="ExternalInput")
    wt = nc.dram_tensor("wt", [C, KH * KW * F], dt_mm, kind="ExternalInput")
    bb = nc.dram_tensor("bb", [F, 1], mybir.dt.float32, kind="ExternalInput")
    out = nc.dram_tensor("out", [NLOC, F, H, W], mybir.dt.float32, kind="ExternalOutput")
    xp_ap, wt_ap, out_ap = xp.ap(), wt.ap(), out.ap()

    f32 = mybir.dt.float32
    x_sb = [nc.alloc_sbuf_tensor(f"x{n}", [C, PH, PW], dt_mm) for n in range(NLOC)]
    wt_sb = nc.alloc_sbuf_tensor("wts", [C, KH * KW * F], dt_mm)
    b_sb = nc.alloc_sbuf_tensor("bs", [F, 1], f32)
    warm_sb = nc.alloc_sbuf_tensor("warms", [C, F], f32)
    o_sb = [nc.alloc_sbuf_tensor(f"o{c}", [F, CH * W], f32) for c in range(2 * NLOC)]
    ps = [nc.alloc_psum_tensor(f"ps{c}", [F, CH * W], f32) for c in range(2 * NLOC)]

    chunks = [(n, h) for n in range(NLOC) for h in range(NCHUNK)]

    with ExitStack() as ctx:
        block = ctx.enter_context(nc.Block())
        b_sem = ctx.enter_context(nc.semaphore("b_sem"))
        w_sem = ctx.enter_context(nc.semaphore("w_sem"))
        ms_sem = ctx.enter_context(nc.semaphore("ms_sem"))
        mm_sem = ctx.enter_context(nc.semaphore("mm_sem"))
        dve_sem = ctx.enter_context(nc.semaphore("dve_sem"))
        od_sem = ctx.enter_context(nc.semaphore("od_sem"))
        xs_sems = [
            ctx.enter_context(nc.semaphore(f"xs{j}")) for j in range(2 * NLOC)
        ]

        @block.sync
        def _(sync):
            # SP HWDGE ring: input images (critical path)
            for n in range(NLOC):
                sync.dma_start(
                    x_sb[n].ap()[:, 0 : CH + 2, :], xp_ap[n, :, 0 : CH + 2, :]
                ).then_inc(xs_sems[2 * n], 16)
                sync.dma_start(
                    x_sb[n].ap()[:, CH + 2 : PH, :], xp_ap[n, :, CH + 2 : PH, :]
                ).then_inc(xs_sems[2 * n + 1], 16)
            sync.wait_ge(od_sem, 16 * 2 * NLOC)

        @block.scalar
        def _(scalar):
            # ACT HWDGE ring: weights + bias in, outputs back
            scalar.dma_start(b_sb.ap(), bb.ap()).then_inc(b_sem, 16)
            for t in range(KH * KW):
                scalar.dma_start(
                    wt_sb.ap()[:, t * F : (t + 1) * F],
                    wt_ap[:, t * F : (t + 1) * F],
                ).then_inc(w_sem, 16)
            for c, (n, h) in enumerate(chunks):
                scalar.wait_ge(dve_sem, c + 1)
                scalar.dma_start(
                    out_ap[n, :, h * CH : (h + 1) * CH, :], o_sb[c].ap()
                ).then_inc(od_sem, 16)

        @block.tensor
        def _(tensor):
            tensor.wait_ge(ms_sem, 1)
            for j in range(N_WARMUP_MM):
                nc.tensor.matmul(
                    ps[0].ap()[:, 0:F],
                    warm_sb.ap(),
                    warm_sb.ap(),
                    start=(j == 0),
                    stop=(j == N_WARMUP_MM - 1),
                )
            tensor.wait_ge(w_sem, 16 * KH * KW)
            for c, (n, h) in enumerate(chunks):
                tensor.wait_ge(xs_sems[2 * n], 16)
                if h == 1:
                    tensor.wait_ge(xs_sems[2 * n + 1], 16)
                x3 = x_sb[n].ap()
                t = 0
                for dy in range(KH):
                    for dx in range(KW):
                        mm = nc.tensor.matmul(
                            ps[c].ap(),
                            wt_sb.ap()[:, t * F : (t + 1) * F],
                            x3[:, h * CH + dy : h * CH + dy + CH, dx : dx + W],
                            start=(t == 0),
                            stop=(t == KH * KW - 1),
                        )
                        t += 1
                mm.then_inc(mm_sem, 1)

        @block.vector
        def _(vector):
            nc.vector.memset(warm_sb.ap(), 0.0).then_inc(ms_sem, 1)
            vector.wait_ge(b_sem, 16)
            for c in range(2 * NLOC):
                vector.wait_ge(mm_sem, c + 1)
                nc.vector.tensor_scalar_add(
                    o_sb[c].ap(), ps[c].ap(), b_sb.ap()
                ).then_inc(dve_sem, 1)

        # reset all sems so a re-execution of the same loaded NEFF starts clean
        all_sems = [b_sem, w_sem, ms_sem, mm_sem, dve_sem, od_sem] + xs_sems
        nums = sorted(s.num for s in all_sems)
        assert nums == list(range(nums[0], nums[0] + len(nums)))
        rng = range(nums[0], nums[-1] + 1)

        @block.gpsimd
        def _(gp):
            gp.wait_ge(b_sem, 16)
            gp.wait_ge(w_sem, 16 * KH * KW)
            gp.wait_ge(ms_sem, 1)
            for j in range(2 * NLOC):
                gp.wait_ge(xs_sems[j], 16)
            gp.wait_ge(mm_sem, 2 * NLOC)
            gp.wait_ge(dve_sem, 2 * NLOC)
            gp.wait_ge(od_sem, 16 * 2 * NLOC)
            gp.dma_reset(rng)
            gp.sem_clear(rng)

    nc.compile()
    return nc


def _np_mm_dtype():
    if MM_DT == "bf16":
        import ml_dtypes

        return np.dtype(ml_dtypes.bfloat16)
    if MM_DT == "fp16":
        return np.dtype(np.float16)
    return np.dtype(np.float32)


RAW = bool(int(os.environ.get("CONV_RAW", "0")))


def prep_inputs(x, w, b):
    np_dt = _np_mm_dtype()
    x = np.asarray(x, dtype=np.float32)
    w = np.asarray(w, dtype=np.float32)
    b = np.asarray(b, dtype=np.float32)

    xp = np.zeros((N, C, PH, PW), dtype=np_dt)
    xp[:, :, 1 : 1 + H, 1 : 1 + W] = x
    # wt[c, (dy*KW+dx)*F + f] = w[f, c, dy, dx]
    wt = np.ascontiguousarray(w.transpose(1, 2, 3, 0)).reshape(C, KH * KW * F)
    wt = wt.astype(np_dt)
    bb = np.ascontiguousarray(b.reshape(F, 1))

    return [
        {"xp": xp[i * NLOC : (i + 1) * NLOC], "wt": wt, "bb": bb}
        for i in range(N_CORES)
    ]


def post_outputs(per_core):
    return np.concatenate([per_core["out"][i] for i in range(N_CORES)], axis=0)


def kernel(x: np.ndarray, w: np.ndarray, b: np.ndarray) -> np.ndarray:
    global _cached_nc, LAST_RESULT
    if _cached_nc is None:
        _cached_nc = _build_raw() if RAW else _build()
    nc = _cached_nc

    in_maps = prep_inputs(x, w, b)
    res = bass_utils.run_bass_kernel_spmd(
        nc,
        in_maps,
        list(range(N_CORES)),
        trace=bool(int(os.environ.get("CONV_TRACE", "0"))),
    )
    LAST_RESULT = res
    return post_outputs({"out": np.stack([r["out"] for r in res.results])})



# revision 18
# speedup vs baseline: 1.2285x; 1.2285x over previous
"""Conv2dfft forward on 8 TRN2 NeuronCores.

The reference computes cross-correlation via rfft2/irfft2 on a 65x65 grid.
Because the FFT grid (65) >= padded_H + KH - 1 (34 + 3 - 1 = 36 would wrap,
but the output is cropped to out_H=32 and max input row touched is 33 < 65),
no circular wraparound reaches the cropped output: the result is EXACTLY a
3x3 same-padding cross-correlation (DL conv, padding=1) plus bias.

So: direct conv as 9 PSUM-accumulated matmuls per output tile.
  out[f, (y,x)] = sum_t sum_c wT[c, t, f] * xpad[c, y+dy_t, x+dx_t]
with contraction over C=128 on the partition dim, in fp16 (HW rel err
3.2e-4; fp8 was measured numerically at 4e-2 > tolerance, Winograd loses
on this chip because DVE/ACT are ~160x weaker than the PE).

Sharding: data-parallel over batch N=32 -> 4 images per core.

Default schedule (CONV_SCHED=v2, HW-tuned by skip/tiny A-B sweeps):
  - PE: per image, 2-3 row-chunks x 9 LDW+MM pairs, FD=512 (one PSUM bank),
    72-90 MMs/body. HW microbench: a bare LDW+MM stream runs ~228ns/MM
    regardless of operand striding, weight identity, or accumulation
    grouping -> the PE stream itself is at the floor (~16.4us incl. loop
    barrier); all baseline overhead (27.7 -> 16.4) was DMA-path serial
    time, not PE inefficiency.
  - DMA: one whole-image input DMA per image (SP ring), one whole-image
    output DMA per image (ACT ring, 4 ops/body instead of 10 chunk-level
    ops). DVE bias-adds write into a per-image [F,H,W] tile.
  - Timing loop: UNROLL=4 bodies per For_i iteration, scheduled as one
    dependency graph -> body k+1's input DMAs prefetch under body k's MMs
    and the ramp/tail/all-engine-barrier amortize by 1/4. The single-shot
    grader path emits exactly one body.
Measured (loop-delta, per conv body): ~16.1us in the device's fast power
state, ~19.9us sustained-throttled, vs 27.7us for the session baseline
measured contemporaneously (device throttles ~20% under back-to-back load;
test.py idles ~10s between reps to stay in steady state).
"""

import os
from contextlib import ExitStack

import numpy as np

import concourse.bacc as bacc
import concourse.mybir as mybir
import concourse.tile as tile
from concourse import bass_utils

N_CORES = 8
N, C, H, W = 32, 128, 32, 32
F = 128
KH = KW = 3
PH, PW = H + 2, W + 2          # padded input 34x34
NLOC = N // N_CORES            # images per core
# output rows per chunk; 16 -> CH*W = 512 = one full PSUM bank
CH = int(os.environ.get("CONV_CH", "16"))
NCHUNK = H // CH

# matmul input dtype for x and w. fp16 runs the PE at full rate (1 cycle/row,
# like bf16) with a 10-bit mantissa: HW-measured rel err 3.2e-4 vs the fp32
# reference. ("f32r" = 1.4e-4 but ~6% slower; "f32" exact but 4x slower.)
MM_DT = os.environ.get("CONV_MM_DT", "fp16")

# Number of times the compute body is emitted in the NEFF (timing tool:
# per-iteration HW time = (t_K - t_1) / (K - 1), host-RPC overhead cancels).
N_ITERS = int(os.environ.get("CONV_ITERS", "1"))
# Bodies per For_i iteration in timing mode. Unrolled bodies are scheduled as
# one dependency graph, so body k+1's input DMAs prefetch during body k's
# matmuls and the ramp/tail/loop-barrier amortize 1/UNROLL. The single-shot
# (grader) path always emits exactly ONE body; the timing loop runs
# N_ITERS * UNROLL bodies and test.py divides by that count.
UNROLL = int(os.environ.get("CONV_UNROLL", "4"))

# v2 pipelining: split input DMAs into row-pieces + PE warmup matmuls.
# Default OFF: HWDGE processes DMA ops on a ring FIFO with ~1us of
# per-op completion serialization, so 9 small pieces gate the PE ~5us
# worse than 4 whole-image DMAs (HW-measured: 25.0us -> 19.9us).
V2 = bool(int(os.environ.get("CONV_V2", "0")))
N_WARMUP_MM = int(os.environ.get("CONV_WARMUP", "4"))

# diagnostics: selectively disable pipeline stages (timing only — results wrong)
SKIP_IN = bool(int(os.environ.get("CONV_SKIP_IN", "0")))
SKIP_OUT = bool(int(os.environ.get("CONV_SKIP_OUT", "0")))
SKIP_MM = bool(int(os.environ.get("CONV_SKIP_MM", "0")))
# split the weight DMA into one piece per tap so the first matmul only waits
# for tap 0 (32KB) instead of the whole 295KB weight block
WSPLIT = bool(int(os.environ.get("CONV_WSPLIT", "0")))
# alternate output DMAs between the ACT and SP HWDGE rings (SP is idle once
# the input stream finishes, so this halves the per-ring output load)
ORING = bool(int(os.environ.get("CONV_ORING", "0")))
# contiguous-rhs schedule: keep x in SBUF as a flat [C, PH*PW] tile and
# stream whole padded rows (rows*34 cols incl. 2 junk cols/row) per matmul.
# Every tap is then a pure start-offset into one flat contiguous AP (no
# nested row-strided access pattern on the PE moving operand); DVE copies
# out only the 32 valid columns per row.
CONTIG = bool(int(os.environ.get("CONV_CONTIG", "0")))

_DT_MAP = {
    "f32r": mybir.dt.float32r,
    "f32": mybir.dt.float32,
    "bf16": mybir.dt.bfloat16,
    "fp16": mybir.dt.float16,
}

# flat schedule: treat the padded image as a flat [C, PH*PW] buffer. Because
# the padded width (34) is >= W + KW - 1 relative to the cropped 32-wide
# output, a 3x3 conv over rows equals a 1D conv over the flat buffer with tap
# offsets dy*PW+dx. The matmul moving operand is then a fully CONTIGUOUS flat
# slice (stride-1, no nested row AP); 2 junk output cols per row are skipped
# by the DVE bias-add copy. Costs ~6% extra moving cols, removes any
# strided-AP penalty on the PE stream.
SCHED = os.environ.get("CONV_SCHED", "v2")
XL = PH * PW + 4  # flat image length, padded so the last tap window fits
_rsplit = os.environ.get("CONV_RSPLIT", "11,11,10")
RSPLIT = [int(r) for r in _rsplit.split(",")]
assert sum(RSPLIT) == H
# tap-outer matmul order (LDW once per tap per image, consecutive same-weight
# MMs) vs chunk-outer (9 distinct-weight MMs per chunk)
TAPOUT = bool(int(os.environ.get("CONV_TAPOUT", "0")))
# split the last image's output DMA per-chunk to shorten the serial tail
TAILSPLIT = bool(int(os.environ.get("CONV_TAILSPLIT", "0")))

_cached_nc = None
LAST_RESULT = None


def _build_v2():
    """v2 schedule: same 16-row chunk matmuls as the baseline, but with a
    coarser DMA plan — the HW evidence (skip/tiny sweeps) says per-op DMA
    serial cost, not bytes, dominates the 27.7us baseline:
      - input:  one whole-block DMA per body (CONV_IN_GRAN=block) or one per
        image (=image), double-buffered across bodies
      - output: one whole-image DMA per image (4/body) instead of 10
        chunk-level DMAs; DVE bias-adds write into a per-image [F,H,W] tile
    CONV_TINY_IN/OUT shrink transfers to 1 row (timing diagnostics only).
    """
    dt_mm = _DT_MAP[MM_DT]
    nc = bacc.Bacc(
        "TRN2",
        target_bir_lowering=False,
        debug=False,
        num_devices=N_CORES,
    )

    IN_GRAN = os.environ.get("CONV_IN_GRAN", "image")
    TINY_IN = bool(int(os.environ.get("CONV_TINY_IN", "0")))
    TINY_OUT = bool(int(os.environ.get("CONV_TINY_OUT", "0")))
    LOOPWARM = int(os.environ.get("CONV_LOOPWARM", "0"))
    TAPER2 = bool(int(os.environ.get("CONV_TAPER", "1")))

    if IN_GRAN == "block":
        xp = nc.dram_tensor("xp", [C, NLOC * PH * PW], dt_mm, kind="ExternalInput")
    else:
        xp = nc.dram_tensor("xp", [NLOC, C, PH, PW], dt_mm, kind="ExternalInput")
    wt = nc.dram_tensor("wt", [C, KH * KW * F], dt_mm, kind="ExternalInput")
    bb = nc.dram_tensor("bb", [F, 1], mybir.dt.float32, kind="ExternalInput")
    out = nc.dram_tensor("out", [NLOC, F, H, W], mybir.dt.float32, kind="ExternalOutput")

    xp_ap = xp.ap()
    out_ap = out.ap()

    with ExitStack() as ctx:
        tc = ctx.enter_context(tile.TileContext(nc))
        const_pool = ctx.enter_context(tc.tile_pool(name="const", bufs=1))
        if IN_GRAN == "block":
            xbufs_default = 2
        else:
            xbufs_default = 8
        x_pool = ctx.enter_context(
            tc.tile_pool(
                name="xs", bufs=int(os.environ.get("CONV_XBUFS", str(xbufs_default)))
            )
        )
        n_ps = int(os.environ.get("CONV_PSBUFS", "8" if LOOPWARM == 0 else "7"))
        ps_pool = ctx.enter_context(tc.tile_pool(name="ps", bufs=n_ps, space="PSUM"))
        o_pool = ctx.enter_context(
            tc.tile_pool(name="os", bufs=int(os.environ.get("CONV_OBUFS", "4")))
        )

        wt_sb = const_pool.tile([C, KH * KW * F], dt_mm)
        nc.scalar.dma_start(wt_sb[:], wt.ap())
        b_sb = const_pool.tile([F, 1], mybir.dt.float32)
        nc.scalar.dma_start(b_sb[:], bb.ap())

        wps_pool = warm_sb = warm_out = None
        if LOOPWARM > 0:
            wps_pool = ctx.enter_context(tc.tile_pool(name="wps", bufs=1, space="PSUM"))
            warm_sb = const_pool.tile([C, F], mybir.dt.float32)
            nc.vector.memset(warm_sb[:], 0.0)
            warm_out = const_pool.tile([F, 1], mybir.dt.float32)

        if TINY_IN:
            # pre-zero the rotating x buffers once so garbage reads are finite
            pre = []
            for i in range(x_pool.bufs):
                if IN_GRAN == "block":
                    t = x_pool.tile([C, NLOC, PH, PW], dt_mm, tag="xs", name=f"pz{i}")
                else:
                    t = x_pool.tile([C, PH, PW], dt_mm, tag="xs", name=f"pz{i}")
                nc.vector.memset(t[:], 0.0)
                pre.append(t)

        def image_chunks(n):
            if not TAPER2:
                return [(0, CH)] + [(y, CH) for y in range(CH, H, CH)]
            if n == 0:
                return [(0, 8), (8, 8), (16, 16)]
            if n == NLOC - 1:
                return [(0, 16), (16, 8), (24, 8)]
            return [(0, 16), (16, 16)]

        def body():
            if LOOPWARM > 0:
                wps = wps_pool.tile([F, F], mybir.dt.float32)
                for j in range(LOOPWARM):
                    nc.tensor.matmul(
                        wps[:], warm_sb[:], warm_sb[:],
                        start=(j == 0), stop=(j == LOOPWARM - 1),
                    )
                nc.vector.tensor_copy(warm_out[:], wps[:, 0:1])

            x_blk = None
            if IN_GRAN == "block":
                x_blk = x_pool.tile([C, NLOC, PH, PW], dt_mm, tag="xs")
                if TINY_IN:
                    nc.sync.dma_start(
                        x_blk[:, 0, 0:1, :], xp_ap[:, 0 : PH * PW][:, 0:PW]
                    )
                else:
                    nc.sync.dma_start(x_blk[:], xp_ap[:, :])

            for n in range(NLOC):
                if IN_GRAN == "block":
                    x_img = x_blk[:, n]
                else:
                    x_img = x_pool.tile([C, PH, PW], dt_mm, tag="xs", name=f"xi{n}")
                    if TINY_IN:
                        nc.sync.dma_start(x_img[:, 0:1, :], xp_ap[n, :, 0:1, :])
                    else:
                        nc.sync.dma_start(x_img[:], xp_ap[n, :, :, :])
                o_img = o_pool.tile([F, H, W], mybir.dt.float32, tag="os", name=f"oi{n}")
                for y0, rows in image_chunks(n):
                    ps = ps_pool.tile(
                        [F, rows, W], mybir.dt.float32, tag="ps", name=f"ps{n}_{y0}"
                    )
                    t = 0
                    for dy in range(KH):
                        for dx in range(KW):
                            nc.tensor.matmul(
                                ps[:],
                                wt_sb[:, t * F : (t + 1) * F],
                                x_img[:, y0 + dy : y0 + dy + rows, dx : dx + W],
                                start=(t == 0),
                                stop=(t == KH * KW - 1),
                            )
                            t += 1
                    nc.vector.tensor_scalar_add(
                        o_img[:, y0 : y0 + rows, :], ps[:], b_sb[:]
                    )
                if TINY_OUT:
                    nc.scalar.dma_start(out_ap[n, :, 0:1, :], o_img[:, 0:1, :])
                else:
                    nc.scalar.dma_start(out_ap[n, :, :, :], o_img[:])

        unroll = UNROLL
        if N_ITERS == 1:
            body()
        else:
            stag = bool(int(os.environ.get("CONV_STAGGER", "0")))
            with tc.For_i(0, N_ITERS, 1, staggered_reset=stag):
                for _ in range(unroll):
                    body()

    nc.compile()
    return nc


def _build_flat():
    """Flat-1D-conv schedule: contiguous moving operands, whole-image output
    DMAs, per-image row chunks RSPLIT (default 11/11/10)."""
    dt_mm = _DT_MAP[MM_DT]
    nc = bacc.Bacc(
        "TRN2",
        target_bir_lowering=False,
        debug=False,
        num_devices=N_CORES,
    )

    xp = nc.dram_tensor("xp", [NLOC, C, XL], dt_mm, kind="ExternalInput")
    wt = nc.dram_tensor("wt", [C, KH * KW * F], dt_mm, kind="ExternalInput")
    bb = nc.dram_tensor("bb", [F, 1], mybir.dt.float32, kind="ExternalInput")
    out = nc.dram_tensor("out", [NLOC, F, H, W], mybir.dt.float32, kind="ExternalOutput")

    xp_ap = xp.ap()
    out_ap = out.ap()
    LOOPWARM = int(os.environ.get("CONV_LOOPWARM", "0"))

    with ExitStack() as ctx:
        tc = ctx.enter_context(tile.TileContext(nc))
        const_pool = ctx.enter_context(tc.tile_pool(name="const", bufs=1))
        x_pool = ctx.enter_context(
            tc.tile_pool(name="xs", bufs=int(os.environ.get("CONV_XBUFS", "4")))
        )
        n_ps = int(os.environ.get("CONV_PSBUFS", "8" if LOOPWARM == 0 else "7"))
        ps_pool = ctx.enter_context(tc.tile_pool(name="ps", bufs=n_ps, space="PSUM"))
        o_pool = ctx.enter_context(
            tc.tile_pool(name="os", bufs=int(os.environ.get("CONV_OBUFS", "3")))
        )

        wt_sb = const_pool.tile([C, KH * KW * F], dt_mm)
        nc.scalar.dma_start(wt_sb[:], wt.ap())
        b_sb = const_pool.tile([F, 1], mybir.dt.float32)
        nc.scalar.dma_start(b_sb[:], bb.ap())

        wps_pool = warm_sb = warm_out = None
        if LOOPWARM > 0:
            wps_pool = ctx.enter_context(tc.tile_pool(name="wps", bufs=1, space="PSUM"))
            warm_sb = const_pool.tile([C, F], mybir.dt.float32)
            nc.vector.memset(warm_sb[:], 0.0)
            warm_out = const_pool.tile([F, 1], mybir.dt.float32)

        y0s = []
        acc = 0
        for r in RSPLIT:
            y0s.append(acc)
            acc += r

        def body():
            if LOOPWARM > 0:
                wps = wps_pool.tile([F, F], mybir.dt.float32)
                for j in range(LOOPWARM):
                    nc.tensor.matmul(
                        wps[:], warm_sb[:], warm_sb[:],
                        start=(j == 0), stop=(j == LOOPWARM - 1),
                    )
                nc.vector.tensor_copy(warm_out[:], wps[:, 0:1])
            for n in range(NLOC):
                x_sb = x_pool.tile([C, XL], dt_mm, tag="xs")
                nc.sync.dma_start(x_sb[:], xp_ap[n, :, :])
                o_img = o_pool.tile([F, H, W], mybir.dt.float32, tag="os")
                ps_tiles = [
                    ps_pool.tile(
                        [F, r, PW], mybir.dt.float32, tag="ps", name=f"ps{n}_{c}"
                    )
                    for c, r in enumerate(RSPLIT)
                ]
                if TAPOUT:
                    for t in range(KH * KW):
                        dy, dx = t // KW, t % KW
                        off = dy * PW + dx
                        for c, (y0, r) in enumerate(zip(y0s, RSPLIT)):
                            nc.tensor.matmul(
                                ps_tiles[c][:],
                                wt_sb[:, t * F : (t + 1) * F],
                                x_sb[:, y0 * PW + off : (y0 + r) * PW + off],
                                start=(t == 0),
                                stop=(t == KH * KW - 1),
                            )
                else:
                    for c, (y0, r) in enumerate(zip(y0s, RSPLIT)):
                        for t in range(KH * KW):
                            dy, dx = t // KW, t % KW
                            off = dy * PW + dx
                            nc.tensor.matmul(
                                ps_tiles[c][:],
                                wt_sb[:, t * F : (t + 1) * F],
                                x_sb[:, y0 * PW + off : (y0 + r) * PW + off],
                                start=(t == 0),
                                stop=(t == KH * KW - 1),
                            )
                for c, (y0, r) in enumerate(zip(y0s, RSPLIT)):
                    nc.vector.tensor_scalar_add(
                        o_img[:, y0 : y0 + r, :], ps_tiles[c][:, :, 0:W], b_sb[:]
                    )
                if TAILSPLIT and n == NLOC - 1:
                    for c, (y0, r) in enumerate(zip(y0s, RSPLIT)):
                        eng = nc.sync if c % 2 == 1 else nc.scalar
                        eng.dma_start(
                            out_ap[n, :, y0 : y0 + r, :], o_img[:, y0 : y0 + r, :]
                        )
                else:
                    nc.scalar.dma_start(out_ap[n, :, :, :], o_img[:])

        unroll = UNROLL
        if N_ITERS == 1:
            for _ in range(unroll):
                body()
        else:
            with tc.For_i(0, N_ITERS, 1):
                for _ in range(unroll):
                    body()

    nc.compile()
    return nc


def _build():
    dt_mm = _DT_MAP[MM_DT]
    nc = bacc.Bacc(
        "TRN2",
        target_bir_lowering=False,
        debug=False,
        num_devices=N_CORES,
    )

    xp = nc.dram_tensor("xp", [NLOC, C, PH, PW], dt_mm, kind="ExternalInput")
    wt = nc.dram_tensor("wt", [C, KH * KW * F], dt_mm, kind="ExternalInput")
    bb = nc.dram_tensor("bb", [F, 1], mybir.dt.float32, kind="ExternalInput")
    out = nc.dram_tensor("out", [NLOC, F, H, W], mybir.dt.float32, kind="ExternalOutput")

    xp_ap = xp.ap()
    out_ap = out.ap()

    if bool(int(os.environ.get("CONV_NULL", "0"))):
        # dispatch-floor measurement: one tiny DMA through SBUF
        with ExitStack() as ctx:
            tc = ctx.enter_context(tile.TileContext(nc))
            np_pool = ctx.enter_context(tc.tile_pool(name="np", bufs=1))
            t_sb = np_pool.tile([F, 1], mybir.dt.float32)
            nc.sync.dma_start(t_sb[:], bb.ap())
            nc.sync.dma_start(out_ap[0, :, 0:1, 0:1], t_sb[:])
        nc.compile()
        return nc

    with ExitStack() as ctx:
        tc = ctx.enter_context(tile.TileContext(nc))
        const_pool = ctx.enter_context(tc.tile_pool(name="const", bufs=1))
        x_pool = ctx.enter_context(
            tc.tile_pool(name="xs", bufs=int(os.environ.get("CONV_XBUFS", "4")))
        )
        _loopwarm = int(os.environ.get("CONV_LOOPWARM", "0"))
        # one PSUM bank is 512 fp32; a [F, CH, W] tile needs CH*W/512 banks
        _ps_fit = max(1, min(8, (8 * 512) // (CH * W)))
        n_ps_bufs = (_ps_fit - 1) if ((V2 and N_WARMUP_MM > 0) or _loopwarm > 0) else _ps_fit
        n_ps_bufs = int(os.environ.get("CONV_PSBUFS", str(n_ps_bufs)))
        ps_pool = ctx.enter_context(tc.tile_pool(name="ps", bufs=n_ps_bufs, space="PSUM"))
        o_pool = ctx.enter_context(
            tc.tile_pool(name="os", bufs=int(os.environ.get("CONV_OBUFS", "5")))
        )

        # weights + bias arrive via the ACT HWDGE ring so they don't delay
        # the image stream on the SP ring
        wt_sb = const_pool.tile([C, KH * KW * F], dt_mm)
        if WSPLIT:
            for t in range(KH * KW):
                nc.scalar.dma_start(
                    wt_sb[:, t * F : (t + 1) * F], wt.ap()[:, t * F : (t + 1) * F]
                )
        else:
            nc.scalar.dma_start(wt_sb[:], wt.ap())
        b_sb = const_pool.tile([F, 1], mybir.dt.float32)
        nc.scalar.dma_start(b_sb[:], bb.ap())

        LOOPWARM = _loopwarm
        wps_pool = warm_sb = warm_out = None
        if (V2 and N_WARMUP_MM > 0) or LOOPWARM > 0:
            # HAM warmup: keep PE busy while the first input DMAs land.
            wps_pool = ctx.enter_context(
                tc.tile_pool(name="wps", bufs=1, space="PSUM")
            )
            warm_sb = const_pool.tile([C, F], mybir.dt.float32)
            nc.vector.memset(warm_sb[:], 0.0)
            warm_out = const_pool.tile([F, 1], mybir.dt.float32)
        if V2 and N_WARMUP_MM > 0:
            wps = wps_pool.tile([F, F], mybir.dt.float32)
            for j in range(N_WARMUP_MM):
                nc.tensor.matmul(
                    wps[:],
                    warm_sb[:],
                    warm_sb[:],
                    start=(j == 0),
                    stop=(j == N_WARMUP_MM - 1),
                )
            nc.vector.tensor_copy(warm_out[:], wps[:, 0:1])

        TAPER = bool(int(os.environ.get("CONV_TAPER", "1")))

        XCONST = bool(int(os.environ.get("CONV_XCONST", "0")))
        IRING = int(os.environ.get("CONV_IRING", "1"))
        RAMPSPLIT = bool(int(os.environ.get("CONV_RAMPSPLIT", "0")))

        x_const = None
        if SKIP_IN or XCONST:
            # PE-isolation probe: all chunks read one memset tile; no input
            # DMAs inside the timed loop
            x_const = const_pool.tile([C, PH, PW], dt_mm)
            nc.vector.memset(x_const[:], 0.0)

        def image_plan(n):
            # (out-row chunks, input DMA row-splits). Tapered: the very first
            # chunk is 8 rows (compute starts after a third of image 0) and
            # the very last is 8 rows (shorter serial drain tail).
            if CH != 16:
                # generic chunking probe: vary matmul granularity while
                # keeping the input DMA split fixed at 18+16 rows
                chunks = [(i * CH, CH) for i in range(H // CH)]
                return chunks, [(0, 18), (18, PH)]
            if TAPER and n == 0:
                return [(0, 8), (8, 8), (16, 16)], [(0, 10), (10, 18), (18, PH)]
            if TAPER and n == NLOC - 1:
                return [(0, 16), (16, 8), (24, 8)], [(0, CH + 2), (CH + 2, PH)]
            return [(0, CH), (CH, CH)], [(0, CH + 2), (CH + 2, PH)]

        INTERLEAVE = bool(int(os.environ.get("CONV_INTERLEAVE", "0")))
        chunk_counter = [0]

        def emit_chunk(x_sb, n, y0, rows):
            if SKIP_MM:
                return
            ci = chunk_counter[0]
            chunk_counter[0] += 1
            ps = ps_pool.tile([F, rows, W], mybir.dt.float32, tag="ps")
            t = 0
            for dy in range(KH):
                for dx in range(KW):
                    rhs = x_sb[:, y0 + dy : y0 + dy + rows, dx : dx + W]
                    nc.tensor.matmul(
                        ps[:],
                        wt_sb[:, t * F : (t + 1) * F],
                        rhs,
                        start=(t == 0),
                        stop=(t == KH * KW - 1),
                    )
                    t += 1
            o_sb = o_pool.tile([F, rows, W], mybir.dt.float32, tag="os")
            nc.vector.tensor_scalar_add(o_sb[:], ps[:], b_sb[:])
            if not SKIP_OUT:
                eng = nc.sync if (ORING and ci % 2 == 1) else nc.scalar
                eng.dma_start(out_ap[n, :, y0 : y0 + rows, :], o_sb[:])

        def body():
            if LOOPWARM > 0:
                # per-iteration PE filler: covers the image-0 DMA ramp so the
                # PE has no idle window at the loop back-edge
                wps = wps_pool.tile([F, F], mybir.dt.float32)
                for j in range(LOOPWARM):
                    nc.tensor.matmul(
                        wps[:],
                        warm_sb[:],
                        warm_sb[:],
                        start=(j == 0),
                        stop=(j == LOOPWARM - 1),
                    )
                nc.vector.tensor_copy(warm_out[:], wps[:, 0:1])
            if not (V2 and INTERLEAVE):
                for n in range(NLOC):
                    chunks_n, dma_splits = image_plan(n)
                    if SKIP_IN:
                        for y0, rows in chunks_n:
                            emit_chunk(x_const, n, y0, rows)
                        continue
                    x_sb = x_pool.tile([C, PH, PW], dt_mm, tag="xs")
                    if V2:
                        for pi, (r0, r1) in enumerate(dma_splits):
                            eng = nc.scalar if (IRING == 2 and pi % 2) else nc.sync
                            eng.dma_start(
                                x_sb[:, r0:r1, :], xp_ap[n, :, r0:r1, :]
                            )
                    else:
                        eng = nc.scalar if (IRING == 2 and n % 2) else nc.sync
                        if RAMPSPLIT and n == 0:
                            # split image 0 so the PE can start ~0.8us earlier
                            eng.dma_start(
                                x_sb[:, 0:18, :], xp_ap[n, :, 0:18, :]
                            )
                            eng.dma_start(
                                x_sb[:, 18:PH, :], xp_ap[n, :, 18:PH, :]
                            )
                        else:
                            eng.dma_start(x_sb[:], xp_ap[n, :, :, :])
                    for y0, rows in chunks_n:
                        emit_chunk(x_const if XCONST else x_sb, n, y0, rows)
                return

            # interleaved schedule: issue DMA pieces in the order chunks need
            # them and interleave chunk compute across images, so the PE works
            # on image n+1's first chunk while image n's tail DMA lands.
            x_tiles = [
                x_pool.tile([C, PH, PW], dt_mm, tag="xs", name=f"xt{i}")
                for i in range(NLOC)
            ]
            plans = [image_plan(n) for n in range(NLOC)]
            # (image, piece) DMA order / (image, chunk) compute order for
            # NLOC=4 with the tapered plans (3,2,2,3 chunks; 3,2,2,2 pieces)
            dma_order = [
                (0, 0), (0, 1), (1, 0), (0, 2), (1, 1),
                (2, 0), (2, 1), (3, 0), (3, 1),
            ]
            chunk_order = [
                (0, 0), (0, 1), (1, 0), (0, 2), (1, 1),
                (2, 0), (2, 1), (3, 0), (3, 1), (3, 2),
            ]
            emitted = set()
            ci = 0
            for di, (n, p) in enumerate(dma_order):
                r0, r1 = plans[n][1][p]
                nc.sync.dma_start(
                    x_tiles[n][:, r0:r1, :], xp_ap[n, :, r0:r1, :]
                )
                emitted.add((n, p))
                # emit every chunk whose input pieces have all been issued
                while ci < len(chunk_order):
                    cn, cc = chunk_order[ci]
                    y0, rows = plans[cn][0][cc]
                    need = [
                        pi
                        for pi, (pr0, pr1) in enumerate(plans[cn][1])
                        if pr0 < y0 + rows + 2 and pr1 > y0
                    ]
                    if not all((cn, pi) in emitted for pi in need):
                        break
                    emit_chunk(x_tiles[cn], cn, y0, rows)
                    ci += 1
            while ci < len(chunk_order):
                cn, cc = chunk_order[ci]
                y0, rows = plans[cn][0][cc]
                emit_chunk(x_tiles[cn], cn, y0, rows)
                ci += 1

        unroll = UNROLL
        if N_ITERS == 1:
            for _ in range(unroll):
                body()
        else:
            # timing mode: run the body N_ITERS*UNROLL times on-device
            stag = bool(int(os.environ.get("CONV_STAGGER", "0")))
            with tc.For_i(0, N_ITERS, 1, staggered_reset=stag):
                for _ in range(unroll):
                    body()

    nc.compile()
    return nc


def _build_raw():
    """Raw-Bacc variant: hand-rolled semaphores, no Tile framework.

    Exactly 8 output chunks per core -> each chunk owns one PSUM bank, all
    SBUF statically allocated (no reuse, no WAR hazards). Avoids the Tile
    kernel-tail drain + all-engine barrier.
    """
    dt_mm = _DT_MAP[MM_DT]
    nc = bacc.Bacc(
        "TRN2",
        target_bir_lowering=False,
        debug=False,
        num_devices=N_CORES,
    )

    xp = nc.dram_tensor("xp", [NLOC, C, PH, PW], dt_mm, kind="ExternalInput")
    wt = nc.dram_tensor("wt", [C, KH * KW * F], dt_mm, kind="ExternalInput")
    bb = nc.dram_tensor("bb", [F, 1], mybir.dt.float32, kind="ExternalInput")
    out = nc.dram_tensor("out", [NLOC, F, H, W], mybir.dt.float32, kind="ExternalOutput")
    xp_ap, wt_ap, out_ap = xp.ap(), wt.ap(), out.ap()

    f32 = mybir.dt.float32
    x_sb = [nc.alloc_sbuf_tensor(f"x{n}", [C, PH, PW], dt_mm) for n in range(NLOC)]
    wt_sb = nc.alloc_sbuf_tensor("wts", [C, KH * KW * F], dt_mm)
    b_sb = nc.alloc_sbuf_tensor("bs", [F, 1], f32)
    warm_sb = nc.alloc_sbuf_tensor("warms", [C, F], f32)
    o_sb = [nc.alloc_sbuf_tensor(f"o{c}", [F, CH * W], f32) for c in range(2 * NLOC)]
    ps = [nc.alloc_psum_tensor(f"ps{c}", [F, CH * W], f32) for c in range(2 * NLOC)]

    chunks = [(n, h) for n in range(NLOC) for h in range(NCHUNK)]

    with ExitStack() as ctx:
        block = ctx.enter_context(nc.Block())
        b_sem = ctx.enter_context(nc.semaphore("b_sem"))
        w_sem = ctx.enter_context(nc.semaphore("w_sem"))
        ms_sem = ctx.enter_context(nc.semaphore("ms_sem"))
        mm_sem = ctx.enter_context(nc.semaphore("mm_sem"))
        dve_sem = ctx.enter_context(nc.semaphore("dve_sem"))
        od_sem = ctx.enter_context(nc.semaphore("od_sem"))
        xs_sems = [
            ctx.enter_context(nc.semaphore(f"xs{j}")) for j in range(2 * NLOC)
        ]

        @block.sync
        def _(sync):
            # SP HWDGE ring: input images (critical path)
            for n in range(NLOC):
                sync.dma_start(
                    x_sb[n].ap()[:, 0 : CH + 2, :], xp_ap[n, :, 0 : CH + 2, :]
                ).then_inc(xs_sems[2 * n], 16)
                sync.dma_start(
                    x_sb[n].ap()[:, CH + 2 : PH, :], xp_ap[n, :, CH + 2 : PH, :]
                ).then_inc(xs_sems[2 * n + 1], 16)
            sync.wait_ge(od_sem, 16 * 2 * NLOC)

        @block.scalar
        def _(scalar):
            # ACT HWDGE ring: weights + bias in, outputs back
            scalar.dma_start(b_sb.ap(), bb.ap()).then_inc(b_sem, 16)
            for t in range(KH * KW):
                scalar.dma_start(
                    wt_sb.ap()[:, t * F : (t + 1) * F],
                    wt_ap[:, t * F : (t + 1) * F],
                ).then_inc(w_sem, 16)
            for c, (n, h) in enumerate(chunks):
                scalar.wait_ge(dve_sem, c + 1)
                scalar.dma_start(
                    out_ap[n, :, h * CH : (h + 1) * CH, :], o_sb[c].ap()
                ).then_inc(od_sem, 16)

        @block.tensor
        def _(tensor):
            tensor.wait_ge(ms_sem, 1)
            for j in range(N_WARMUP_MM):
                nc.tensor.matmul(
                    ps[0].ap()[:, 0:F],
                    warm_sb.ap(),
                    warm_sb.ap(),
                    start=(j == 0),
                    stop=(j == N_WARMUP_MM - 1),
                )
            tensor.wait_ge(w_sem, 16 * KH * KW)
            for c, (n, h) in enumerate(chunks):
                tensor.wait_ge(xs_sems[2 * n], 16)
                if h == 1:
                    tensor.wait_ge(xs_sems[2 * n + 1], 16)
                x3 = x_sb[n].ap()
                t = 0
                for dy in range(KH):
                    for dx in range(KW):
                        mm = nc.tensor.matmul(
                            ps[c].ap(),
                            wt_sb.ap()[:, t * F : (t + 1) * F],
                            x3[:, h * CH + dy : h * CH + dy + CH, dx : dx + W],
                            start=(t == 0),
                            stop=(t == KH * KW - 1),
                        )
                        t += 1
                mm.then_inc(mm_sem, 1)

        @block.vector
        def _(vector):
            nc.vector.memset(warm_sb.ap(), 0.0).then_inc(ms_sem, 1)
            vector.wait_ge(b_sem, 16)
            for c in range(2 * NLOC):
                vector.wait_ge(mm_sem, c + 1)
                nc.vector.tensor_scalar_add(
                    o_sb[c].ap(), ps[c].ap(), b_sb.ap()
                ).then_inc(dve_sem, 1)

        # reset all sems so a re-execution of the same loaded NEFF starts clean
        all_sems = [b_sem, w_sem, ms_sem, mm_sem, dve_sem, od_sem] + xs_sems
        nums = sorted(s.num for s in all_sems)
        assert nums == list(range(nums[0], nums[0] + len(nums)))
        rng = range(nums[0], nums[-1] + 1)

        @block.gpsimd
        def _(gp):
            gp.wait_ge(b_sem, 16)
            gp.wait_ge(w_sem, 16 * KH * KW)
            gp.wait_ge(ms_sem, 1)
            for j in range(2 * NLOC):
                gp.wait_ge(xs_sems[j], 16)
            gp.wait_ge(mm_sem, 2 * NLOC)
            gp.wait_ge(dve_sem, 2 * NLOC)
            gp.wait_ge(od_sem, 16 * 2 * NLOC)
            gp.dma_reset(rng)
            gp.sem_clear(rng)

    nc.compile()
    return nc


def _np_mm_dtype():
    if MM_DT == "bf16":
        import ml_dtypes

        return np.dtype(ml_dtypes.bfloat16)
    if MM_DT == "fp16":
        return np.dtype(np.float16)
    return np.dtype(np.float32)


RAW = bool(int(os.environ.get("CONV_RAW", "0")))


def build_nc():
    if bool(int(os.environ.get("CONV_NULL", "0"))):
        # dispatch-floor null kernel (timing reference) lives in _build()
        return _build()
    if RAW:
        return _build_raw()
    if SCHED == "flat":
        return _build_flat()
    if SCHED == "v2":
        return _build_v2()
    return _build()


def prep_inputs(x, w, b):
    np_dt = _np_mm_dtype()
    x = np.asarray(x, dtype=np.float32)
    w = np.asarray(w, dtype=np.float32)
    b = np.asarray(b, dtype=np.float32)

    if SCHED == "flat":
        xp = np.zeros((N, C, XL), dtype=np_dt)
        pad = np.zeros((N, C, PH, PW), dtype=np_dt)
        pad[:, :, 1 : 1 + H, 1 : 1 + W] = x
        xp[:, :, : PH * PW] = pad.reshape(N, C, PH * PW)
    else:
        xp = np.zeros((N, C, PH, PW), dtype=np_dt)
        xp[:, :, 1 : 1 + H, 1 : 1 + W] = x
    if SCHED == "v2" and os.environ.get("CONV_IN_GRAN", "image") == "block":
        # per-core [C, NLOC*PH*PW] block layout for the single input DMA
        wt_np = np.ascontiguousarray(w.transpose(1, 2, 3, 0)).reshape(C, KH * KW * F)
        wt_np = wt_np.astype(np_dt)
        bb_np = np.ascontiguousarray(b.reshape(F, 1))
        return [
            {
                "xp": np.ascontiguousarray(
                    xp[i * NLOC : (i + 1) * NLOC].transpose(1, 0, 2, 3)
                ).reshape(C, NLOC * PH * PW),
                "wt": wt_np,
                "bb": bb_np,
            }
            for i in range(N_CORES)
        ]
    # wt[c, (dy*KW+dx)*F + f] = w[f, c, dy, dx]
    wt = np.ascontiguousarray(w.transpose(1, 2, 3, 0)).reshape(C, KH * KW * F)
    wt = wt.astype(np_dt)
    bb = np.ascontiguousarray(b.reshape(F, 1))

    return [
        {"xp": xp[i * NLOC : (i + 1) * NLOC], "wt": wt, "bb": bb}
        for i in range(N_CORES)
    ]


def post_outputs(per_core):
    return np.concatenate([per_core["out"][i] for i in range(N_CORES)], axis=0)


def kernel(x: np.ndarray, w: np.ndarray, b: np.ndarray) -> np.ndarray:
    global _cached_nc, LAST_RESULT
    if _cached_nc is None:
        _cached_nc = build_nc()
    nc = _cached_nc

    in_maps = prep_inputs(x, w, b)
    res = bass_utils.run_bass_kernel_spmd(
        nc,
        in_maps,
        list(range(N_CORES)),
        trace=bool(int(os.environ.get("CONV_TRACE", "0"))),
    )
    LAST_RESULT = res
    return post_outputs({"out": np.stack([r["out"] for r in res.results])})



# revision 20
# speedup vs baseline: 1.2959x; 1.0549x over previous
"""Conv2dfft forward on 8 TRN2 NeuronCores.

The reference computes cross-correlation via rfft2/irfft2 on a 65x65 grid.
Because the FFT grid (65) >= padded_H + KH - 1 (34 + 3 - 1 = 36 would wrap,
but the output is cropped to out_H=32 and max input row touched is 33 < 65),
no circular wraparound reaches the cropped output: the result is EXACTLY a
3x3 same-padding cross-correlation (DL conv, padding=1) plus bias.

So: direct conv as 9 PSUM-accumulated matmuls per output tile.
  out[f, (y,x)] = sum_t sum_c wT[c, t, f] * xpad[c, y+dy_t, x+dx_t]
with contraction over C=128 on the partition dim, in fp16 (HW rel err
3.2e-4; fp8 was measured numerically at 4e-2 > tolerance, Winograd loses
on this chip because DVE/ACT are ~160x weaker than the PE).

Sharding: data-parallel over batch N=32 -> 4 images per core.

Default schedule (CONV_SCHED=v2, HW-tuned by skip/tiny A-B sweeps):
  - PE: per image, 2-3 row-chunks x 9 LDW+MM pairs, FD=512 (one PSUM bank),
    72-90 MMs/body. HW microbench: a bare LDW+MM stream runs ~228ns/MM
    regardless of operand striding, weight identity, or accumulation
    grouping -> the PE stream itself is at the floor (~16.4us incl. loop
    barrier); all baseline overhead (27.7 -> 16.4) was DMA-path serial
    time, not PE inefficiency.
  - DMA: one whole-image input DMA per image (SP ring), one whole-image
    output DMA per image (ACT ring, 4 ops/body instead of 10 chunk-level
    ops). DVE bias-adds write into a per-image [F,H,W] tile.
  - Timing loop: UNROLL=4 bodies per For_i iteration, scheduled as one
    dependency graph -> body k+1's input DMAs prefetch under body k's MMs
    and the ramp/tail/all-engine-barrier amortize by 1/4. The single-shot
    grader path emits exactly one body.
Measured (loop-delta, per conv body): ~16.1us in the device's fast power
state, ~19.9us sustained-throttled, vs 27.7us for the session baseline
measured contemporaneously (device throttles ~20% under back-to-back load;
test.py idles ~10s between reps to stay in steady state).
"""

import os
from contextlib import ExitStack

import numpy as np

import concourse.bacc as bacc
import concourse.mybir as mybir
import concourse.tile as tile
from concourse import bass_utils

N_CORES = 8
N, C, H, W = 32, 128, 32, 32
F = 128
KH = KW = 3
PH, PW = H + 2, W + 2          # padded input 34x34
NLOC = N // N_CORES            # images per core
# output rows per chunk; 16 -> CH*W = 512 = one full PSUM bank
CH = int(os.environ.get("CONV_CH", "16"))
NCHUNK = H // CH

# matmul input dtype for x and w. fp16 runs the PE at full rate (1 cycle/row,
# like bf16) with a 10-bit mantissa: HW-measured rel err 3.2e-4 vs the fp32
# reference. ("f32r" = 1.4e-4 but ~6% slower; "f32" exact but 4x slower.)
MM_DT = os.environ.get("CONV_MM_DT", "fp16")

# Number of times the compute body is emitted in the NEFF (timing tool:
# per-iteration HW time = (t_K - t_1) / (K - 1), host-RPC overhead cancels).
N_ITERS = int(os.environ.get("CONV_ITERS", "1"))
# Bodies per For_i iteration in timing mode. Unrolled bodies are scheduled as
# one dependency graph, so body k+1's input DMAs prefetch during body k's
# matmuls and the ramp/tail/loop-barrier amortize 1/UNROLL. The single-shot
# (grader) path always emits exactly ONE body; the timing loop runs
# N_ITERS * UNROLL bodies and test.py divides by that count.
UNROLL = int(os.environ.get("CONV_UNROLL", "4"))

# v2 pipelining: split input DMAs into row-pieces + PE warmup matmuls.
# Default OFF: HWDGE processes DMA ops on a ring FIFO with ~1us of
# per-op completion serialization, so 9 small pieces gate the PE ~5us
# worse than 4 whole-image DMAs (HW-measured: 25.0us -> 19.9us).
V2 = bool(int(os.environ.get("CONV_V2", "0")))
N_WARMUP_MM = int(os.environ.get("CONV_WARMUP", "4"))

# diagnostics: selectively disable pipeline stages (timing only — results wrong)
SKIP_IN = bool(int(os.environ.get("CONV_SKIP_IN", "0")))
SKIP_OUT = bool(int(os.environ.get("CONV_SKIP_OUT", "0")))
SKIP_MM = bool(int(os.environ.get("CONV_SKIP_MM", "0")))
# split the weight DMA into one piece per tap so the first matmul only waits
# for tap 0 (32KB) instead of the whole 295KB weight block
WSPLIT = bool(int(os.environ.get("CONV_WSPLIT", "0")))
# alternate output DMAs between the ACT and SP HWDGE rings (SP is idle once
# the input stream finishes, so this halves the per-ring output load)
ORING = bool(int(os.environ.get("CONV_ORING", "1")))
# contiguous-rhs schedule: keep x in SBUF as a flat [C, PH*PW] tile and
# stream whole padded rows (rows*34 cols incl. 2 junk cols/row) per matmul.
# Every tap is then a pure start-offset into one flat contiguous AP (no
# nested row-strided access pattern on the PE moving operand); DVE copies
# out only the 32 valid columns per row.
CONTIG = bool(int(os.environ.get("CONV_CONTIG", "0")))

_DT_MAP = {
    "f32r": mybir.dt.float32r,
    "f32": mybir.dt.float32,
    "bf16": mybir.dt.bfloat16,
    "fp16": mybir.dt.float16,
}

# flat schedule: treat the padded image as a flat [C, PH*PW] buffer. Because
# the padded width (34) is >= W + KW - 1 relative to the cropped 32-wide
# output, a 3x3 conv over rows equals a 1D conv over the flat buffer with tap
# offsets dy*PW+dx. The matmul moving operand is then a fully CONTIGUOUS flat
# slice (stride-1, no nested row AP); 2 junk output cols per row are skipped
# by the DVE bias-add copy. Costs ~6% extra moving cols, removes any
# strided-AP penalty on the PE stream.
SCHED = os.environ.get("CONV_SCHED", "v2")
XL = PH * PW + 4  # flat image length, padded so the last tap window fits
_rsplit = os.environ.get("CONV_RSPLIT", "11,11,10")
RSPLIT = [int(r) for r in _rsplit.split(",")]
assert sum(RSPLIT) == H
# tap-outer matmul order (LDW once per tap per image, consecutive same-weight
# MMs) vs chunk-outer (9 distinct-weight MMs per chunk)
TAPOUT = bool(int(os.environ.get("CONV_TAPOUT", "0")))
# split the last image's output DMA per-chunk to shorten the serial tail
TAILSPLIT = bool(int(os.environ.get("CONV_TAILSPLIT", "0")))

_cached_nc = None
LAST_RESULT = None


def _build_v2():
    """v2 schedule: same 16-row chunk matmuls as the baseline, but with a
    coarser DMA plan — the HW evidence (skip/tiny sweeps) says per-op DMA
    serial cost, not bytes, dominates the 27.7us baseline:
      - input:  one whole-block DMA per body (CONV_IN_GRAN=block) or one per
        image (=image), double-buffered across bodies
      - output: one whole-image DMA per image (4/body) instead of 10
        chunk-level DMAs; DVE bias-adds write into a per-image [F,H,W] tile
    CONV_TINY_IN/OUT shrink transfers to 1 row (timing diagnostics only).
    """
    dt_mm = _DT_MAP[MM_DT]
    nc = bacc.Bacc(
        "TRN2",
        target_bir_lowering=False,
        debug=False,
        num_devices=N_CORES,
    )

    IN_GRAN = os.environ.get("CONV_IN_GRAN", "image")
    TINY_IN = bool(int(os.environ.get("CONV_TINY_IN", "0")))
    TINY_OUT = bool(int(os.environ.get("CONV_TINY_OUT", "0")))
    LOOPWARM = int(os.environ.get("CONV_LOOPWARM", "0"))
    TAPER2 = bool(int(os.environ.get("CONV_TAPER", "1")))

    if IN_GRAN == "block":
        xp = nc.dram_tensor("xp", [C, NLOC * PH * PW], dt_mm, kind="ExternalInput")
    else:
        xp = nc.dram_tensor("xp", [NLOC, C, PH, PW], dt_mm, kind="ExternalInput")
    wt = nc.dram_tensor("wt", [C, KH * KW * F], dt_mm, kind="ExternalInput")
    bb = nc.dram_tensor("bb", [F, 1], mybir.dt.float32, kind="ExternalInput")
    out = nc.dram_tensor("out", [NLOC, F, H, W], mybir.dt.float32, kind="ExternalOutput")

    xp_ap = xp.ap()
    out_ap = out.ap()

    with ExitStack() as ctx:
        tc = ctx.enter_context(tile.TileContext(nc))
        const_pool = ctx.enter_context(tc.tile_pool(name="const", bufs=1))
        if IN_GRAN == "block":
            xbufs_default = 2
        else:
            xbufs_default = 8
        x_pool = ctx.enter_context(
            tc.tile_pool(
                name="xs", bufs=int(os.environ.get("CONV_XBUFS", str(xbufs_default)))
            )
        )
        n_ps = int(os.environ.get("CONV_PSBUFS", "8" if LOOPWARM == 0 else "7"))
        ps_pool = ctx.enter_context(tc.tile_pool(name="ps", bufs=n_ps, space="PSUM"))
        o_pool = ctx.enter_context(
            tc.tile_pool(name="os", bufs=int(os.environ.get("CONV_OBUFS", "4")))
        )

        wt_sb = const_pool.tile([C, KH * KW * F], dt_mm)
        nc.scalar.dma_start(wt_sb[:], wt.ap())
        b_sb = const_pool.tile([F, 1], mybir.dt.float32)
        nc.scalar.dma_start(b_sb[:], bb.ap())

        wps_pool = warm_sb = warm_out = None
        if LOOPWARM > 0:
            wps_pool = ctx.enter_context(tc.tile_pool(name="wps", bufs=1, space="PSUM"))
            warm_sb = const_pool.tile([C, F], mybir.dt.float32)
            nc.vector.memset(warm_sb[:], 0.0)
            warm_out = const_pool.tile([F, 1], mybir.dt.float32)

        if TINY_IN:
            # pre-zero the rotating x buffers once so garbage reads are finite
            pre = []
            for i in range(x_pool.bufs):
                if IN_GRAN == "block":
                    t = x_pool.tile([C, NLOC, PH, PW], dt_mm, tag="xs", name=f"pz{i}")
                else:
                    t = x_pool.tile([C, PH, PW], dt_mm, tag="xs", name=f"pz{i}")
                nc.vector.memset(t[:], 0.0)
                pre.append(t)

        def image_chunks(n):
            if not TAPER2:
                return [(0, CH)] + [(y, CH) for y in range(CH, H, CH)]
            if n == 0:
                return [(0, 8), (8, 8), (16, 16)]
            if n == NLOC - 1:
                return [(0, 16), (16, 8), (24, 8)]
            return [(0, 16), (16, 16)]

        def body():
            if LOOPWARM > 0:
                wps = wps_pool.tile([F, F], mybir.dt.float32)
                for j in range(LOOPWARM):
                    nc.tensor.matmul(
                        wps[:], warm_sb[:], warm_sb[:],
                        start=(j == 0), stop=(j == LOOPWARM - 1),
                    )
                nc.vector.tensor_copy(warm_out[:], wps[:, 0:1])

            x_blk = None
            if IN_GRAN == "block":
                x_blk = x_pool.tile([C, NLOC, PH, PW], dt_mm, tag="xs")
                if TINY_IN:
                    nc.sync.dma_start(
                        x_blk[:, 0, 0:1, :], xp_ap[:, 0 : PH * PW][:, 0:PW]
                    )
                else:
                    nc.sync.dma_start(x_blk[:], xp_ap[:, :])

            for n in range(NLOC):
                if IN_GRAN == "block":
                    x_img = x_blk[:, n]
                else:
                    x_img = x_pool.tile([C, PH, PW], dt_mm, tag="xs", name=f"xi{n}")
                    if TINY_IN:
                        nc.sync.dma_start(x_img[:, 0:1, :], xp_ap[n, :, 0:1, :])
                    else:
                        nc.sync.dma_start(x_img[:], xp_ap[n, :, :, :])
                o_img = o_pool.tile([F, H, W], mybir.dt.float32, tag="os", name=f"oi{n}")
                for y0, rows in image_chunks(n):
                    ps = ps_pool.tile(
                        [F, rows, W], mybir.dt.float32, tag="ps", name=f"ps{n}_{y0}"
                    )
                    t = 0
                    for dy in range(KH):
                        for dx in range(KW):
                            nc.tensor.matmul(
                                ps[:],
                                wt_sb[:, t * F : (t + 1) * F],
                                x_img[:, y0 + dy : y0 + dy + rows, dx : dx + W],
                                start=(t == 0),
                                stop=(t == KH * KW - 1),
                            )
                            t += 1
                    nc.vector.tensor_scalar_add(
                        o_img[:, y0 : y0 + rows, :], ps[:], b_sb[:]
                    )
                if TINY_OUT:
                    nc.scalar.dma_start(out_ap[n, :, 0:1, :], o_img[:, 0:1, :])
                else:
                    # ORING: alternate output DMAs between the ACT and SP
                    # HWDGE rings to halve per-ring completion serialization
                    eng = nc.sync if (ORING and n % 2 == 1) else nc.scalar
                    eng.dma_start(out_ap[n, :, :, :], o_img[:])

        unroll = UNROLL
        if N_ITERS == 1:
            body()
        else:
            stag = bool(int(os.environ.get("CONV_STAGGER", "0")))
            with tc.For_i(0, N_ITERS, 1, staggered_reset=stag):
                for _ in range(unroll):
                    body()

    nc.compile()
    return nc


def _build_flat():
    """Flat-1D-conv schedule: contiguous moving operands, whole-image output
    DMAs, per-image row chunks RSPLIT (default 11/11/10)."""
    dt_mm = _DT_MAP[MM_DT]
    nc = bacc.Bacc(
        "TRN2",
        target_bir_lowering=False,
        debug=False,
        num_devices=N_CORES,
    )

    xp = nc.dram_tensor("xp", [NLOC, C, XL], dt_mm, kind="ExternalInput")
    wt = nc.dram_tensor("wt", [C, KH * KW * F], dt_mm, kind="ExternalInput")
    bb = nc.dram_tensor("bb", [F, 1], mybir.dt.float32, kind="ExternalInput")
    out = nc.dram_tensor("out", [NLOC, F, H, W], mybir.dt.float32, kind="ExternalOutput")

    xp_ap = xp.ap()
    out_ap = out.ap()
    LOOPWARM = int(os.environ.get("CONV_LOOPWARM", "0"))

    with ExitStack() as ctx:
        tc = ctx.enter_context(tile.TileContext(nc))
        const_pool = ctx.enter_context(tc.tile_pool(name="const", bufs=1))
        x_pool = ctx.enter_context(
            tc.tile_pool(name="xs", bufs=int(os.environ.get("CONV_XBUFS", "4")))
        )
        n_ps = int(os.environ.get("CONV_PSBUFS", "8" if LOOPWARM == 0 else "7"))
        ps_pool = ctx.enter_context(tc.tile_pool(name="ps", bufs=n_ps, space="PSUM"))
        o_pool = ctx.enter_context(
            tc.tile_pool(name="os", bufs=int(os.environ.get("CONV_OBUFS", "3")))
        )

        wt_sb = const_pool.tile([C, KH * KW * F], dt_mm)
        nc.scalar.dma_start(wt_sb[:], wt.ap())
        b_sb = const_pool.tile([F, 1], mybir.dt.float32)
        nc.scalar.dma_start(b_sb[:], bb.ap())

        wps_pool = warm_sb = warm_out = None
        if LOOPWARM > 0:
            wps_pool = ctx.enter_context(tc.tile_pool(name="wps", bufs=1, space="PSUM"))
            warm_sb = const_pool.tile([C, F], mybir.dt.float32)
            nc.vector.memset(warm_sb[:], 0.0)
            warm_out = const_pool.tile([F, 1], mybir.dt.float32)

        y0s = []
        acc = 0
        for r in RSPLIT:
            y0s.append(acc)
            acc += r

        def body():
            if LOOPWARM > 0:
                wps = wps_pool.tile([F, F], mybir.dt.float32)
                for j in range(LOOPWARM):
                    nc.tensor.matmul(
                        wps[:], warm_sb[:], warm_sb[:],
                        start=(j == 0), stop=(j == LOOPWARM - 1),
                    )
                nc.vector.tensor_copy(warm_out[:], wps[:, 0:1])
            for n in range(NLOC):
                x_sb = x_pool.tile([C, XL], dt_mm, tag="xs")
                nc.sync.dma_start(x_sb[:], xp_ap[n, :, :])
                o_img = o_pool.tile([F, H, W], mybir.dt.float32, tag="os")
                ps_tiles = [
                    ps_pool.tile(
                        [F, r, PW], mybir.dt.float32, tag="ps", name=f"ps{n}_{c}"
                    )
                    for c, r in enumerate(RSPLIT)
                ]
                if TAPOUT:
                    for t in range(KH * KW):
                        dy, dx = t // KW, t % KW
                        off = dy * PW + dx
                        for c, (y0, r) in enumerate(zip(y0s, RSPLIT)):
                            nc.tensor.matmul(
                                ps_tiles[c][:],
                                wt_sb[:, t * F : (t + 1) * F],
                                x_sb[:, y0 * PW + off : (y0 + r) * PW + off],
                                start=(t == 0),
                                stop=(t == KH * KW - 1),
                            )
                else:
                    for c, (y0, r) in enumerate(zip(y0s, RSPLIT)):
                        for t in range(KH * KW):
                            dy, dx = t // KW, t % KW
                            off = dy * PW + dx
                            nc.tensor.matmul(
                                ps_tiles[c][:],
                                wt_sb[:, t * F : (t + 1) * F],
                                x_sb[:, y0 * PW + off : (y0 + r) * PW + off],
                                start=(t == 0),
                                stop=(t == KH * KW - 1),
                            )
                for c, (y0, r) in enumerate(zip(y0s, RSPLIT)):
                    nc.vector.tensor_scalar_add(
                        o_img[:, y0 : y0 + r, :], ps_tiles[c][:, :, 0:W], b_sb[:]
                    )
                if TAILSPLIT and n == NLOC - 1:
                    for c, (y0, r) in enumerate(zip(y0s, RSPLIT)):
                        eng = nc.sync if c % 2 == 1 else nc.scalar
                        eng.dma_start(
                            out_ap[n, :, y0 : y0 + r, :], o_img[:, y0 : y0 + r, :]
                        )
                else:
                    nc.scalar.dma_start(out_ap[n, :, :, :], o_img[:])

        unroll = UNROLL
        if N_ITERS == 1:
            for _ in range(unroll):
                body()
        else:
            with tc.For_i(0, N_ITERS, 1):
                for _ in range(unroll):
                    body()

    nc.compile()
    return nc


def _build():
    dt_mm = _DT_MAP[MM_DT]
    nc = bacc.Bacc(
        "TRN2",
        target_bir_lowering=False,
        debug=False,
        num_devices=N_CORES,
    )

    xp = nc.dram_tensor("xp", [NLOC, C, PH, PW], dt_mm, kind="ExternalInput")
    wt = nc.dram_tensor("wt", [C, KH * KW * F], dt_mm, kind="ExternalInput")
    bb = nc.dram_tensor("bb", [F, 1], mybir.dt.float32, kind="ExternalInput")
    out = nc.dram_tensor("out", [NLOC, F, H, W], mybir.dt.float32, kind="ExternalOutput")

    xp_ap = xp.ap()
    out_ap = out.ap()

    if bool(int(os.environ.get("CONV_NULL", "0"))):
        # dispatch-floor measurement: one tiny DMA through SBUF
        with ExitStack() as ctx:
            tc = ctx.enter_context(tile.TileContext(nc))
            np_pool = ctx.enter_context(tc.tile_pool(name="np", bufs=1))
            t_sb = np_pool.tile([F, 1], mybir.dt.float32)
            nc.sync.dma_start(t_sb[:], bb.ap())
            nc.sync.dma_start(out_ap[0, :, 0:1, 0:1], t_sb[:])
        nc.compile()
        return nc

    with ExitStack() as ctx:
        tc = ctx.enter_context(tile.TileContext(nc))
        const_pool = ctx.enter_context(tc.tile_pool(name="const", bufs=1))
        x_pool = ctx.enter_context(
            tc.tile_pool(name="xs", bufs=int(os.environ.get("CONV_XBUFS", "4")))
        )
        _loopwarm = int(os.environ.get("CONV_LOOPWARM", "0"))
        # one PSUM bank is 512 fp32; a [F, CH, W] tile needs CH*W/512 banks
        _ps_fit = max(1, min(8, (8 * 512) // (CH * W)))
        n_ps_bufs = (_ps_fit - 1) if ((V2 and N_WARMUP_MM > 0) or _loopwarm > 0) else _ps_fit
        n_ps_bufs = int(os.environ.get("CONV_PSBUFS", str(n_ps_bufs)))
        ps_pool = ctx.enter_context(tc.tile_pool(name="ps", bufs=n_ps_bufs, space="PSUM"))
        o_pool = ctx.enter_context(
            tc.tile_pool(name="os", bufs=int(os.environ.get("CONV_OBUFS", "5")))
        )

        # weights + bias arrive via the ACT HWDGE ring so they don't delay
        # the image stream on the SP ring
        wt_sb = const_pool.tile([C, KH * KW * F], dt_mm)
        if WSPLIT:
            for t in range(KH * KW):
                nc.scalar.dma_start(
                    wt_sb[:, t * F : (t + 1) * F], wt.ap()[:, t * F : (t + 1) * F]
                )
        else:
            nc.scalar.dma_start(wt_sb[:], wt.ap())
        b_sb = const_pool.tile([F, 1], mybir.dt.float32)
        nc.scalar.dma_start(b_sb[:], bb.ap())

        LOOPWARM = _loopwarm
        wps_pool = warm_sb = warm_out = None
        if (V2 and N_WARMUP_MM > 0) or LOOPWARM > 0:
            # HAM warmup: keep PE busy while the first input DMAs land.
            wps_pool = ctx.enter_context(
                tc.tile_pool(name="wps", bufs=1, space="PSUM")
            )
            warm_sb = const_pool.tile([C, F], mybir.dt.float32)
            nc.vector.memset(warm_sb[:], 0.0)
            warm_out = const_pool.tile([F, 1], mybir.dt.float32)
        if V2 and N_WARMUP_MM > 0:
            wps = wps_pool.tile([F, F], mybir.dt.float32)
            for j in range(N_WARMUP_MM):
                nc.tensor.matmul(
                    wps[:],
                    warm_sb[:],
                    warm_sb[:],
                    start=(j == 0),
                    stop=(j == N_WARMUP_MM - 1),
                )
            nc.vector.tensor_copy(warm_out[:], wps[:, 0:1])

        TAPER = bool(int(os.environ.get("CONV_TAPER", "1")))

        XCONST = bool(int(os.environ.get("CONV_XCONST", "0")))
        IRING = int(os.environ.get("CONV_IRING", "1"))
        RAMPSPLIT = bool(int(os.environ.get("CONV_RAMPSPLIT", "0")))

        x_const = None
        if SKIP_IN or XCONST:
            # PE-isolation probe: all chunks read one memset tile; no input
            # DMAs inside the timed loop
            x_const = const_pool.tile([C, PH, PW], dt_mm)
            nc.vector.memset(x_const[:], 0.0)

        def image_plan(n):
            # (out-row chunks, input DMA row-splits). Tapered: the very first
            # chunk is 8 rows (compute starts after a third of image 0) and
            # the very last is 8 rows (shorter serial drain tail).
            if CH != 16:
                # generic chunking probe: vary matmul granularity while
                # keeping the input DMA split fixed at 18+16 rows
                chunks = [(i * CH, CH) for i in range(H // CH)]
                return chunks, [(0, 18), (18, PH)]
            if TAPER and n == 0:
                return [(0, 8), (8, 8), (16, 16)], [(0, 10), (10, 18), (18, PH)]
            if TAPER and n == NLOC - 1:
                return [(0, 16), (16, 8), (24, 8)], [(0, CH + 2), (CH + 2, PH)]
            return [(0, CH), (CH, CH)], [(0, CH + 2), (CH + 2, PH)]

        INTERLEAVE = bool(int(os.environ.get("CONV_INTERLEAVE", "0")))
        chunk_counter = [0]

        def emit_chunk(x_sb, n, y0, rows):
            if SKIP_MM:
                return
            ci = chunk_counter[0]
            chunk_counter[0] += 1
            ps = ps_pool.tile([F, rows, W], mybir.dt.float32, tag="ps")
            t = 0
            for dy in range(KH):
                for dx in range(KW):
                    rhs = x_sb[:, y0 + dy : y0 + dy + rows, dx : dx + W]
                    nc.tensor.matmul(
                        ps[:],
                        wt_sb[:, t * F : (t + 1) * F],
                        rhs,
                        start=(t == 0),
                        stop=(t == KH * KW - 1),
                    )
                    t += 1
            o_sb = o_pool.tile([F, rows, W], mybir.dt.float32, tag="os")
            nc.vector.tensor_scalar_add(o_sb[:], ps[:], b_sb[:])
            if not SKIP_OUT:
                eng = nc.sync if (ORING and ci % 2 == 1) else nc.scalar
                eng.dma_start(out_ap[n, :, y0 : y0 + rows, :], o_sb[:])

        def body():
            if LOOPWARM > 0:
                # per-iteration PE filler: covers the image-0 DMA ramp so the
                # PE has no idle window at the loop back-edge
                wps = wps_pool.tile([F, F], mybir.dt.float32)
                for j in range(LOOPWARM):
                    nc.tensor.matmul(
                        wps[:],
                        warm_sb[:],
                        warm_sb[:],
                        start=(j == 0),
                        stop=(j == LOOPWARM - 1),
                    )
                nc.vector.tensor_copy(warm_out[:], wps[:, 0:1])
            if not (V2 and INTERLEAVE):
                for n in range(NLOC):
                    chunks_n, dma_splits = image_plan(n)
                    if SKIP_IN:
                        for y0, rows in chunks_n:
                            emit_chunk(x_const, n, y0, rows)
                        continue
                    x_sb = x_pool.tile([C, PH, PW], dt_mm, tag="xs")
                    if V2:
                        for pi, (r0, r1) in enumerate(dma_splits):
                            eng = nc.scalar if (IRING == 2 and pi % 2) else nc.sync
                            eng.dma_start(
                                x_sb[:, r0:r1, :], xp_ap[n, :, r0:r1, :]
                            )
                    else:
                        eng = nc.scalar if (IRING == 2 and n % 2) else nc.sync
                        if RAMPSPLIT and n == 0:
                            # split image 0 so the PE can start ~0.8us earlier
                            eng.dma_start(
                                x_sb[:, 0:18, :], xp_ap[n, :, 0:18, :]
                            )
                            eng.dma_start(
                                x_sb[:, 18:PH, :], xp_ap[n, :, 18:PH, :]
                            )
                        else:
                            eng.dma_start(x_sb[:], xp_ap[n, :, :, :])
                    for y0, rows in chunks_n:
                        emit_chunk(x_const if XCONST else x_sb, n, y0, rows)
                return

            # interleaved schedule: issue DMA pieces in the order chunks need
            # them and interleave chunk compute across images, so the PE works
            # on image n+1's first chunk while image n's tail DMA lands.
            x_tiles = [
                x_pool.tile([C, PH, PW], dt_mm, tag="xs", name=f"xt{i}")
                for i in range(NLOC)
            ]
            plans = [image_plan(n) for n in range(NLOC)]
            # (image, piece) DMA order / (image, chunk) compute order for
            # NLOC=4 with the tapered plans (3,2,2,3 chunks; 3,2,2,2 pieces)
            dma_order = [
                (0, 0), (0, 1), (1, 0), (0, 2), (1, 1),
                (2, 0), (2, 1), (3, 0), (3, 1),
            ]
            chunk_order = [
                (0, 0), (0, 1), (1, 0), (0, 2), (1, 1),
                (2, 0), (2, 1), (3, 0), (3, 1), (3, 2),
            ]
            emitted = set()
            ci = 0
            for di, (n, p) in enumerate(dma_order):
                r0, r1 = plans[n][1][p]
                nc.sync.dma_start(
                    x_tiles[n][:, r0:r1, :], xp_ap[n, :, r0:r1, :]
                )
                emitted.add((n, p))
                # emit every chunk whose input pieces have all been issued
                while ci < len(chunk_order):
                    cn, cc = chunk_order[ci]
                    y0, rows = plans[cn][0][cc]
                    need = [
                        pi
                        for pi, (pr0, pr1) in enumerate(plans[cn][1])
                        if pr0 < y0 + rows + 2 and pr1 > y0
                    ]
                    if not all((cn, pi) in emitted for pi in need):
                        break
                    emit_chunk(x_tiles[cn], cn, y0, rows)
                    ci += 1
            while ci < len(chunk_order):
                cn, cc = chunk_order[ci]
                y0, rows = plans[cn][0][cc]
                emit_chunk(x_tiles[cn], cn, y0, rows)
                ci += 1

        unroll = UNROLL
        if N_ITERS == 1:
            for _ in range(unroll):
                body()
        else:
            # timing mode: run the body N_ITERS*UNROLL times on-device
            stag = bool(int(os.environ.get("CONV_STAGGER", "0")))
            with tc.For_i(0, N_ITERS, 1, staggered_reset=stag):
                for _ in range(unroll):
                    body()

    nc.compile()
    return nc


def _build_raw():
    """Raw-Bacc variant: hand-rolled semaphores, no Tile framework.

    Exactly 8 output chunks per core -> each chunk owns one PSUM bank, all
    SBUF statically allocated (no reuse, no WAR hazards). Avoids the Tile
    kernel-tail drain + all-engine barrier.
    """
    dt_mm = _DT_MAP[MM_DT]
    nc = bacc.Bacc(
        "TRN2",
        target_bir_lowering=False,
        debug=False,
        num_devices=N_CORES,
    )

    xp = nc.dram_tensor("xp", [NLOC, C, PH, PW], dt_mm, kind="ExternalInput")
    wt = nc.dram_tensor("wt", [C, KH * KW * F], dt_mm, kind="ExternalInput")
    bb = nc.dram_tensor("bb", [F, 1], mybir.dt.float32, kind="ExternalInput")
    out = nc.dram_tensor("out", [NLOC, F, H, W], mybir.dt.float32, kind="ExternalOutput")
    xp_ap, wt_ap, out_ap = xp.ap(), wt.ap(), out.ap()

    f32 = mybir.dt.float32
    x_sb = [nc.alloc_sbuf_tensor(f"x{n}", [C, PH, PW], dt_mm) for n in range(NLOC)]
    wt_sb = nc.alloc_sbuf_tensor("wts", [C, KH * KW * F], dt_mm)
    b_sb = nc.alloc_sbuf_tensor("bs", [F, 1], f32)
    warm_sb = nc.alloc_sbuf_tensor("warms", [C, F], f32)
    o_sb = [nc.alloc_sbuf_tensor(f"o{c}", [F, CH * W], f32) for c in range(2 * NLOC)]
    ps = [nc.alloc_psum_tensor(f"ps{c}", [F, CH * W], f32) for c in range(2 * NLOC)]

    chunks = [(n, h) for n in range(NLOC) for h in range(NCHUNK)]

    with ExitStack() as ctx:
        block = ctx.enter_context(nc.Block())
        b_sem = ctx.enter_context(nc.semaphore("b_sem"))
        w_sem = ctx.enter_context(nc.semaphore("w_sem"))
        ms_sem = ctx.enter_context(nc.semaphore("ms_sem"))
        mm_sem = ctx.enter_context(nc.semaphore("mm_sem"))
        dve_sem = ctx.enter_context(nc.semaphore("dve_sem"))
        od_sem = ctx.enter_context(nc.semaphore("od_sem"))
        xs_sems = [
            ctx.enter_context(nc.semaphore(f"xs{j}")) for j in range(2 * NLOC)
        ]

        @block.sync
        def _(sync):
            # SP HWDGE ring: input images (critical path)
            for n in range(NLOC):
                sync.dma_start(
                    x_sb[n].ap()[:, 0 : CH + 2, :], xp_ap[n, :, 0 : CH + 2, :]
                ).then_inc(xs_sems[2 * n], 16)
                sync.dma_start(
                    x_sb[n].ap()[:, CH + 2 : PH, :], xp_ap[n, :, CH + 2 : PH, :]
                ).then_inc(xs_sems[2 * n + 1], 16)
            sync.wait_ge(od_sem, 16 * 2 * NLOC)

        @block.scalar
        def _(scalar):
            # ACT HWDGE ring: weights + bias in, outputs back
            scalar.dma_start(b_sb.ap(), bb.ap()).then_inc(b_sem, 16)
            for t in range(KH * KW):
                scalar.dma_start(
                    wt_sb.ap()[:, t * F : (t + 1) * F],
                    wt_ap[:, t * F : (t + 1) * F],
                ).then_inc(w_sem, 16)
            for c, (n, h) in enumerate(chunks):
                scalar.wait_ge(dve_sem, c + 1)
                scalar.dma_start(
                    out_ap[n, :, h * CH : (h + 1) * CH, :], o_sb[c].ap()
                ).then_inc(od_sem, 16)

        @block.tensor
        def _(tensor):
            tensor.wait_ge(ms_sem, 1)
            for j in range(N_WARMUP_MM):
                nc.tensor.matmul(
                    ps[0].ap()[:, 0:F],
                    warm_sb.ap(),
                    warm_sb.ap(),
                    start=(j == 0),
                    stop=(j == N_WARMUP_MM - 1),
                )
            tensor.wait_ge(w_sem, 16 * KH * KW)
            for c, (n, h) in enumerate(chunks):
                tensor.wait_ge(xs_sems[2 * n], 16)
                if h == 1:
                    tensor.wait_ge(xs_sems[2 * n + 1], 16)
                x3 = x_sb[n].ap()
                t = 0
                for dy in range(KH):
                    for dx in range(KW):
                        mm = nc.tensor.matmul(
                            ps[c].ap(),
                            wt_sb.ap()[:, t * F : (t + 1) * F],
                            x3[:, h * CH + dy : h * CH + dy + CH, dx : dx + W],
                            start=(t == 0),
                            stop=(t == KH * KW - 1),
                        )
                        t += 1
                mm.then_inc(mm_sem, 1)

        @block.vector
        def _(vector):
            nc.vector.memset(warm_sb.ap(), 0.0).then_inc(ms_sem, 1)
            vector.wait_ge(b_sem, 16)
            for c in range(2 * NLOC):
                vector.wait_ge(mm_sem, c + 1)
                nc.vector.tensor_scalar_add(
                    o_sb[c].ap(), ps[c].ap(), b_sb.ap()
                ).then_inc(dve_sem, 1)

        # reset all sems so a re-execution of the same loaded NEFF starts clean
        all_sems = [b_sem, w_sem, ms_sem, mm_sem, dve_sem, od_sem] + xs_sems
        nums = sorted(s.num for s in all_sems)
        assert nums == list(range(nums[0], nums[0] + len(nums)))
        rng = range(nums[0], nums[-1] + 1)

        @block.gpsimd
        def _(gp):
            gp.wait_ge(b_sem, 16)
            gp.wait_ge(w_sem, 16 * KH * KW)
            gp.wait_ge(ms_sem, 1)
            for j in range(2 * NLOC):
                gp.wait_ge(xs_sems[j], 16)
            gp.wait_ge(mm_sem, 2 * NLOC)
            gp.wait_ge(dve_sem, 2 * NLOC)
            gp.wait_ge(od_sem, 16 * 2 * NLOC)
            gp.dma_reset(rng)
            gp.sem_clear(rng)

    nc.compile()
    return nc


def _np_mm_dtype():
    if MM_DT == "bf16":
        import ml_dtypes

        return np.dtype(ml_dtypes.bfloat16)
    if MM_DT == "fp16":
        return np.dtype(np.float16)
    return np.dtype(np.float32)


RAW = bool(int(os.environ.get("CONV_RAW", "0")))


def build_nc():
    if bool(int(os.environ.get("CONV_NULL", "0"))):
        # dispatch-floor null kernel (timing reference) lives in _build()
        return _build()
    if RAW:
        return _build_raw()
    if SCHED == "flat":
        return _build_flat()
    if SCHED == "v2":
        return _build_v2()
    return _build()


def prep_inputs(x, w, b):
    np_dt = _np_mm_dtype()
    x = np.asarray(x, dtype=np.float32)
    w = np.asarray(w, dtype=np.float32)
    b = np.asarray(b, dtype=np.float32)

    if SCHED == "flat":
        xp = np.zeros((N, C, XL), dtype=np_dt)
        pad = np.zeros((N, C, PH, PW), dtype=np_dt)
        pad[:, :, 1 : 1 + H, 1 : 1 + W] = x
        xp[:, :, : PH * PW] = pad.reshape(N, C, PH * PW)
    else:
        xp = np.zeros((N, C, PH, PW), dtype=np_dt)
        xp[:, :, 1 : 1 + H, 1 : 1 + W] = x
    if SCHED == "v2" and os.environ.get("CONV_IN_GRAN", "image") == "block":
        # per-core [C, NLOC*PH*PW] block layout for the single input DMA
        wt_np = np.ascontiguousarray(w.transpose(1, 2, 3, 0)).reshape(C, KH * KW * F)
        wt_np = wt_np.astype(np_dt)
        bb_np = np.ascontiguousarray(b.reshape(F, 1))
        return [
            {
                "xp": np.ascontiguousarray(
                    xp[i * NLOC : (i + 1) * NLOC].transpose(1, 0, 2, 3)
                ).reshape(C, NLOC * PH * PW),
                "wt": wt_np,
                "bb": bb_np,
            }
            for i in range(N_CORES)
        ]
    # wt[c, (dy*KW+dx)*F + f] = w[f, c, dy, dx]
    wt = np.ascontiguousarray(w.transpose(1, 2, 3, 0)).reshape(C, KH * KW * F)
    wt = wt.astype(np_dt)
    bb = np.ascontiguousarray(b.reshape(F, 1))

    return [
        {"xp": xp[i * NLOC : (i + 1) * NLOC], "wt": wt, "bb": bb}
        for i in range(N_CORES)
    ]


def post_outputs(per_core):
    return np.concatenate([per_core["out"][i] for i in range(N_CORES)], axis=0)


def kernel(x: np.ndarray, w: np.ndarray, b: np.ndarray) -> np.ndarray:
    global _cached_nc, LAST_RESULT
    if _cached_nc is None:
        _cached_nc = build_nc()
    nc = _cached_nc

    in_maps = prep_inputs(x, w, b)
    res = bass_utils.run_bass_kernel_spmd(
        nc,
        in_maps,
        list(range(N_CORES)),
        trace=bool(int(os.environ.get("CONV_TRACE", "0"))),
    )
    LAST_RESULT = res
    return post_outputs({"out": np.stack([r["out"] for r in res.results])})

